# revision 34
# baseline (speedup 1.0000x reference)
"""NSA-style compressed + top-k block-sparse attention (MiniCPMSparseFlashAttention2)
for Trainium2, distributed over 8 NeuronCores.

Key reduction (validated against the reference): with KERNEL=32, STRIDE=16,
BLOCK=64, TOPK=4, INIT_BLOCKS=1, LOCAL_BLOCKS=2, the reference top-k selection
collapses to: query block qb attends to key blocks
    qb=0 -> {} (output exactly 0), qb=1 -> {0}, qb=2 -> {0,1},
    qb>=3 -> {0, qb-1, b*} where b* = argmax over b in [1, qb-2] of the
    max-pooled compressed-attention (stage 1) score.

Stage 2 runs fully TRANSPOSED (logits/probs laid out [keys, rows]). The
per-token dynamic K^T and V blocks are fetched with block-granular SWDGE
transpose-gathers (one int16 block index per token, elem = whole 64x128
block): K's natural [key, d] block rows transpose-gather into [d, key, token]
slabs; V uses a host-duplicated [128, 128] block layout so each slab half
carries the full block, letting even/odd tokens supply [64 keys, 128 d]
stationaries on partition halves matched to the quadrant layout of the pair
logits. Softmax denominators come from ones-stationary matmuls; 1/l is
exp(-ln(l)) on the scalar engine; the final [d, row] output is scaled by a
PE-broadcast 1/l and untransposed on the host during unshard.

Sharding: 8 cores = 2 KV heads x 4 query-block interleaves (core part p owns
query blocks p, p+4, ..., p+28 - balanced). One uniform program for all cores.
"""
import sys
sys.path.insert(0, '/opt/trn_rl_repo')
import math
import re as _re
import numpy as np
import ml_dtypes

import concourse.bass as bass
import concourse.tile as tile
import concourse.mybir as mybir
from concourse import masks
from concourse.bass_utils import run_bass_kernel_spmd
from concourse.library_config import mlp
from concourse.library_overlay import lower_extended_insts

dt = mybir.dt
F32 = dt.float32
BF16 = dt.bfloat16
I16 = dt.int16
AF = mybir.ActivationFunctionType
ALU = mybir.AluOpType
AX = mybir.AxisListType

S, HQ, HKV, D = 2048, 16, 2, 128
G = HQ // HKV                     # 8 query heads per kv head
KERNEL, STRIDE, BLOCK = 32, 16, 64
B = S // BLOCK                    # 32 kv blocks
C = (S - KERNEL) // STRIDE + 1    # 127 compressed keys
NCORES = 8
NPART = 4                         # seq-parallel parts per kv head
NQ = S // NPART                   # 512 queries per core
NCH = NQ * G // 128               # 32 chunks of 128 (query,g) rows
NST = NQ // 128                   # 4 score tiles / gather waves
QPC = 16                          # queries per chunk
WCH = NCH // NST                  # 8 chunks per wave
SCALE = 1.0 / math.sqrt(D)
SCALE1 = SCALE / KERNEL           # stage-1 kcmp left unnormalized (sum not mean)
NEG = -1e30


# ---------------------------------------------------------------------------
# Workaround for this container's bass/walrus build: TileContext's exit drain
# carries all end-of-kernel sem waits on one CTRL Drain instruction, which
# this walrus rejects ("Too many sync wait commands"). Emit the waits as
# separate SP wait_ge instructions and a bare drain instead.
def _patched_drain_and_barrier(self, tick_clock, wait_clock):
    nc = self.nc
    ticks = [int(v) for v in _re.findall(r"-?\d+", repr(tick_clock.global_clock))]
    sems = self.sems.allocated()
    for proc, sem in sems.items():
        t = ticks[proc]
        if t > 0:
            nc.sync.wait_ge(sem, t * (16 if "DMA" in sem.name else 1))
    nc.sync.drain()
    nc.all_engine_barrier()
    popped = nc._tile_sem_poison_stack.pop()
    assert popped is self._sem_poison
    nc.clear_and_free_semaphores(list(sems.values()))
    nc.all_engine_barrier()


tile.TileContext._drain_and_barrier = _patched_drain_and_barrier


def _split_excess_waits(nc, keep=1):
    """Walrus here rejects >1 sync wait on several instruction encodings.
    Move excess waits onto injected same-engine InstEventSemaphore
    instructions placed immediately before."""
    for f in nc.m.functions:
        for bb in f.blocks:
            old = list(bb.instructions)
            if not any(i.sync_info and i.sync_info.on_wait and
                       len(i.sync_info.on_wait) > keep for i in old):
                continue
            new = []
            for inst in old:
                si = inst.sync_info
                if si and si.on_wait and len(si.on_wait) > keep:
                    waits = list(si.on_wait)
                    excess, kept = waits[:-keep], waits[-keep:]
                    for w in excess:
                        new.append(mybir.InstEventSemaphore(
                            name=nc.get_next_instruction_name(),
                            engine=inst.engine, ins=[], outs=[],
                            sync_info=mybir.SyncInfo(on_wait=[w], on_update=[]),
                        ))
                    inst.sync_info = mybir.SyncInfo(
                        on_wait=kept, on_update=list(si.on_update))
                new.append(inst)
            bb.instructions = new


def build_program(_for_sim=False):
    # the bigger scratch only widens CoreSim's SWDGE-ring reclaim model;
    # walrus reserves its own fixed carveout for the real NEFF
    nc = bass.Bass("TRN2", num_devices=NCORES,
                   dynamic_dma_scratch_size=32768 if _for_sim else 16384)
    tensors = dict(
        qT=nc.dram_tensor("qT", [128, NQ * G], F32, kind="ExternalInput"),
        qTb=nc.dram_tensor("qTb", [128, NQ * G], BF16, kind="ExternalInput"),
        kT=nc.dram_tensor("kT", [128, S], F32, kind="ExternalInput"),
        kabT=nc.dram_tensor("kabT", [128, NCH * 128], BF16, kind="ExternalInput"),
        vab=nc.dram_tensor("vab", [128, NCH * 128], BF16, kind="ExternalInput"),
        b2a=nc.dram_tensor("b2a", [128, NCH], F32, kind="ExternalInput"),
        b2b=nc.dram_tensor("b2b", [128, NCH], F32, kind="ExternalInput"),
        bias1=nc.dram_tensor("bias1", [128, NCH * C], BF16, kind="ExternalInput"),
        mmid=nc.dram_tensor("mmid", [128, NST * B], F32, kind="ExternalInput"),
        iotab=nc.dram_tensor("iotab", [128, B], F32, kind="ExternalInput"),
        aones48=nc.dram_tensor("aones48", [128, 48], F32, kind="ExternalInput"),
        onesrow=nc.dram_tensor("onesrow", [1, 128], BF16, kind="ExternalInput"),
        onesb=nc.dram_tensor("onesb", [128, 1], BF16, kind="ExternalInput"),
        sel128=nc.dram_tensor("sel128", [128, 128], F32, kind="ExternalInput"),
        qmask=nc.dram_tensor("qmask", [128, 8], F32, kind="ExternalInput"),
        eps20=nc.dram_tensor("eps20", [128, 1], F32, kind="ExternalInput"),
        k16=nc.dram_tensor("k16", [B, 64 * 128], BF16, kind="ExternalInput"),
        v16a=nc.dram_tensor("v16a", [B, 64 * 128], BF16, kind="ExternalInput"),
        v16b=nc.dram_tensor("v16b", [B, 64 * 128], BF16, kind="ExternalInput"),
        out=nc.dram_tensor("out", [128, NCH * 128], F32, kind="ExternalOutput"),
    )
    with tile.TileContext(nc) as tc:
        _build_body(nc, tc, tensors)
    if not _for_sim:
        lower_extended_insts(nc)
        _split_excess_waits(nc)
    return nc


def _build_body(nc, tc, t):
    from contextlib import ExitStack
    with ExitStack() as ctx:
        const = ctx.enter_context(tc.tile_pool(name="const", bufs=1))

        nc.gpsimd.load_library(mlp)

        qT = const.tile([128, NQ * G], F32)
        qTb = const.tile([128, NQ * G], BF16)
        kT = const.tile([128, S], F32)
        kabT = const.tile([128, NCH * 128], BF16)
        vab = const.tile([128, NCH * 128], BF16)
        b2a = const.tile([128, NCH], F32)
        b2b = const.tile([128, NCH], F32)
        bias1 = const.tile([128, NCH * C], BF16)
        mmid = const.tile([128, NST * B], F32)
        iotab = const.tile([128, B], F32)
        aones48 = const.tile([128, 48], F32)
        onesrow = const.tile([1, 128], BF16)
        onesb = const.tile([128, 1], BF16)
        sel128 = const.tile([128, 128], F32)
        qmask = const.tile([128, 8], F32)
        eps20 = const.tile([128, 1], F32)
        for j in range(4):
            nc.sync.dma_start(qT[:, j * 1024:(j + 1) * 1024],
                              t["qT"][:, j * 1024:(j + 1) * 1024])
            nc.sync.dma_start(qTb[:, j * 1024:(j + 1) * 1024],
                              t["qTb"][:, j * 1024:(j + 1) * 1024])
            nc.sync.dma_start(kabT[:, j * 1024:(j + 1) * 1024],
                              t["kabT"][:, j * 1024:(j + 1) * 1024])
            nc.sync.dma_start(vab[:, j * 1024:(j + 1) * 1024],
                              t["vab"][:, j * 1024:(j + 1) * 1024])
            nc.sync.dma_start(bias1[:, j * 1016:(j + 1) * 1016],
                              t["bias1"][:, j * 1016:(j + 1) * 1016])
        nc.sync.dma_start(kT[:], t["kT"][:])
        nc.sync.dma_start(b2a[:], t["b2a"][:])
        nc.sync.dma_start(b2b[:], t["b2b"][:])
        nc.sync.dma_start(mmid[:], t["mmid"][:])
        nc.sync.dma_start(iotab[:], t["iotab"][:])
        nc.sync.dma_start(aones48[:], t["aones48"][:])
        nc.sync.dma_start(onesrow[:], t["onesrow"][:])
        nc.sync.dma_start(onesb[:], t["onesb"][:])
        nc.sync.dma_start(sel128[:], t["sel128"][:])
        nc.sync.dma_start(qmask[:], t["qmask"][:])
        nc.sync.dma_start(eps20[:], t["eps20"][:])
        out_d = t["out"]

        ident = const.tile([128, 128], F32)
        masks.make_identity(nc, ident[:])

        # ---- compressed keys: kcmpT[d, c] = sum_{j<32} kT[d, 16c+j] -------
        half = const.tile([128, 128], F32)
        nc.vector.tensor_copy(half[:], kT[:, 0:S:16])
        for j in range(1, 16):
            nc.vector.tensor_add(half[:], half[:], kT[:, j:S:16])
        kcmpT = const.tile([128, C], F32)
        nc.vector.tensor_add(kcmpT[:], half[:, 0:C], half[:, 1:C + 1])

        e1scr = const.tile([128, C], BF16)  # accumulation-only exp target

        gidx = ctx.enter_context(tc.tile_pool(name="gidx", bufs=2))
        kgp = ctx.enter_context(tc.tile_pool(name="kgp", bufs=2))
        vgp = ctx.enter_context(tc.tile_pool(name="vgp", bufs=2))
        s1 = ctx.enter_context(tc.tile_pool(name="s1", bufs=2))
        s1b = ctx.enter_context(tc.tile_pool(name="s1b", bufs=4))
        s2 = ctx.enter_context(tc.tile_pool(name="s2", bufs=3))
        s2o = ctx.enter_context(tc.tile_pool(name="s2o", bufs=2))
        s2b = ctx.enter_context(tc.tile_pool(name="s2b", bufs=4))

        gathered = {}
        with tc.tile_pool(name="ps_lg1", bufs=3, space="PSUM") as ps_lg1, \
             tc.tile_pool(name="ps_sc", bufs=2, space="PSUM") as ps_sc, \
             tc.tile_pool(name="ps_bt", bufs=2, space="PSUM") as ps_bt:

            # ================= stage 1: scores + argmax block ==============
            score_ps = [None] * NST

            def s1_chunk(ch):
                st, sub = divmod(ch, WCH)
                lg1 = ps_lg1.tile([128, C], F32, tag="lg1")
                nc.tensor.matmul(lg1[:], qT[:, 128 * ch:128 * ch + 128],
                                 kcmpT[:], start=True, stop=True)
                ml = s1.tile([128, C], F32, tag="ml")
                nc.vector.tensor_add(ml[:], lg1[:],
                                     bias1[:, C * ch:C * ch + C])
                l1 = s1b.tile([128, 1], F32, tag="l1")
                nc.scalar.activation(e1scr[:], ml[:], AF.Exp, scale=SCALE1,
                                     accum_out=l1[:])
                nlnl = s1b.tile([128, 1], F32, tag="nlnl")
                nc.scalar.activation(nlnl[:], l1[:], AF.Ln, bias=eps20[:, 0:1])
                nc.scalar.activation(nlnl[:], nlnl[:], AF.Copy, scale=-1.0)
                p1 = s1.tile([128, C], F32, tag="p1")
                nc.scalar.activation(p1[:], ml[:], AF.Exp, scale=SCALE1,
                                     bias=nlnl[:])

                if sub == 0:
                    score_ps[st] = ps_sc.tile([128, 128], F32, tag="score",
                                              name="score")
                j = sub // 2
                if sub % 2 == 0:
                    nc.tensor.matmul(score_ps[st][32 * j:32 * j + 32, 0:C],
                                     aones48[:, 16:48], p1[:],
                                     start=True, stop=False,
                                     tile_position=(0, 32 * j))
                else:
                    nc.tensor.matmul(score_ps[st][32 * j:32 * j + 32, 0:C],
                                     aones48[:, 0:32], p1[:],
                                     start=False, stop=True,
                                     tile_position=(0, 32 * j))
                if sub == WCH - 1:
                    _argmax(score_ps[st], st)

            def _argmax(score, st):
                blk = s1.tile([128, B], F32, tag="blk")
                nc.vector.tensor_copy(blk[:], score[:, 0:125:4])
                nc.vector.tensor_tensor(blk[:], blk[:], score[:, 1:126:4],
                                        op=ALU.max)
                nc.vector.tensor_tensor(blk[:], blk[:], score[:, 2:127:4],
                                        op=ALU.max)
                nc.vector.tensor_tensor(blk[:, 0:31], blk[:, 0:31],
                                        score[:, 3:127:4], op=ALU.max)
                nc.vector.tensor_tensor(blk[:, 1:32], blk[:, 1:32],
                                        score[:, 3:127:4], op=ALU.max)
                nc.vector.tensor_add(blk[:], blk[:], mmid[:, B * st:B * st + B])
                mx = s1b.tile([128, 1], F32, tag="mx")
                nc.vector.tensor_reduce(mx[:], blk[:], axis=AX.X, op=ALU.max)
                enc = s1.tile([128, B], F32, tag="enc")
                nc.vector.tensor_scalar(enc[:], blk[:], mx[:], 1024.0,
                                        op0=ALU.is_lt, op1=ALU.mult)
                nc.vector.tensor_tensor(enc[:], enc[:], iotab[:], op=ALU.add)
                bsf = s1b.tile([128, 1], F32, tag="bsf")
                nc.vector.tensor_reduce(bsf[:], enc[:], axis=AX.X, op=ALU.min)
                # wrapped block indices for the block gathers:
                # ktp[P, s] = bsf[16 s + P%16]
                rq = s1b.tile([128, 8], F32, tag="rq")
                nc.vector.tensor_tensor(rq[:], bsf[:].to_broadcast([128, 8]),
                                        qmask[:], op=ALU.mult)
                ktp = ps_bt.tile([128, 8], F32, tag="ktp")
                nc.tensor.matmul(ktp[:], sel128[:], rq[:], start=True, stop=True)
                idxK = gidx.tile([128, 8], I16, tag="idxK")
                nc.vector.tensor_copy(idxK[:], ktp[:])
                gathered[f"idxK{st}"] = idxK

            def wave_gather(st):
                """Block-granular K^T and V gathers for wave st (1 idx/token)."""
                idxK = gathered[f"idxK{st}"]
                kgT = kgp.tile([128, 64, 128], BF16, tag="kgT")
                nc.gpsimd.dma_gather(kgT[:], t["k16"][:], idxK[:],
                                     128, 128, 64 * 128, transpose=True)
                vg = vgp.tile([128, 128, 128], BF16, tag="vg")
                nc.gpsimd.dma_gather(vg[:, 0:64, :], t["v16a"][:], idxK[:],
                                     128, 128, 64 * 128, transpose=True)
                nc.gpsimd.dma_gather(vg[:, 64:128, :], t["v16b"][:], idxK[:],
                                     128, 128, 64 * 128, transpose=True)
                return kgT, vg

            # stage 1 + gather launches
            for st in range(NST):
                for sub in range(WCH):
                    s1_chunk(WCH * st + sub)
                gathered[st] = wave_gather(st)

        # ================= stage 2: block-sparse attention =================
        with tc.tile_pool(name="psAB", bufs=2, space="PSUM") as psAB, \
             tc.tile_pool(name="psL", bufs=2, space="PSUM") as psL, \
             tc.tile_pool(name="ps_o", bufs=3, space="PSUM") as ps_o:

            def s2_chunk(ch, kgT, vg):
                st, pw = divmod(ch, WCH)
                co = 128 * ch
                # fixed 128 keys (block0 | block qb-1), transposed logits in
                # cols 0:128; dynamic per-token logits in cols 128:256
                lgAB = psAB.tile([128, 256], F32, tag="lgAB")
                nc.tensor.matmul(lgAB[:, 0:128], kabT[:, co:co + 128],
                                 qTb[:, co:co + 128], start=True, stop=True)
                kgv = kgT[:]
                for i16 in range(16):
                    i = 16 * pw + i16               # wave-local token
                    cb = 128 + 8 * i16
                    nc.tensor.matmul(
                        lgAB[0:64, cb:cb + 8],
                        kgv[:, :, i],
                        qTb[:, co + 8 * i16:co + 8 * i16 + 8],
                        start=True, stop=True)
                e2 = s2.tile([128, 256], BF16, tag="e2")
                eA = e2[:][:, 0:128]
                ptB = e2[:][:, 128:256]
                nc.scalar.activation(eA, lgAB[:, 0:128], AF.Exp, scale=SCALE,
                                     bias=b2a[:, ch:ch + 1])
                nc.scalar.activation(ptB[0:64, :], lgAB[0:64, 128:256], AF.Exp,
                                     scale=SCALE, bias=b2b[0:64, ch:ch + 1])
                nc.vector.memset(ptB[64:128, :], 0.0)
                # softmax denominators (column sums) in [0:1, 0:128];
                # 1/l = exp(-ln(l)) on the scalar engine; PE-broadcast to
                # [:, 128:256]
                lr = psL.tile([128, 256], F32, tag="lr")
                nc.tensor.matmul(lr[0:1, 0:128], onesb[:], ptB,
                                 start=True, stop=False)
                nc.tensor.matmul(lr[0:1, 0:128], onesb[:], eA,
                                 start=False, stop=True)
                lnl = s2b.tile([1, 128], F32, tag="lnl")
                nc.scalar.activation(lnl[:], lr[0:1, 0:128], AF.Ln, bias=eps20[0:1, 0:1])
                r2T = s2b.tile([1, 128], BF16, tag="r2T")
                nc.scalar.activation(r2T[:], lnl[:], AF.Exp, scale=-1.0)
                nc.tensor.matmul(lr[:, 128:256], onesrow[:], r2T[:],
                                 start=True, stop=True)
                r2b = s2.tile([128, 128], F32, tag="r2b")
                nc.vector.tensor_copy(r2b[:], lr[:, 128:256])
                # PV accumulation (transposed output [d, row]); per-token
                # stationaries from the duplicated V slabs, row groups matched
                # to the quadrant partition halves
                oT = ps_o.tile([128, 128], F32, tag="oT")
                nc.tensor.matmul(oT[:], vab[:, co:co + 128], eA,
                                 start=True, stop=False)
                for i16 in range(16):
                    i = 16 * pw + i16
                    rc = 8 * i16
                    nc.tensor.matmul(
                        oT[:, rc:rc + 8],
                        vg[:, :, i],
                        ptB[:, rc:rc + 8],
                        start=False, stop=(i16 == 15))
                outc = s2o.tile([128, 128], F32, tag="outc")
                nc.vector.tensor_tensor(outc[:], oT[:], r2b[:], op=ALU.mult)
                nc.sync.dma_start(out_d[:, co:co + 128], outc[:])

            for st in range(NST):
                kgT, vg = gathered[st]
                for sub in range(WCH):
                    s2_chunk(WCH * st + sub, kgT, vg)


_NC_CACHE = None


def _get_program():
    global _NC_CACHE
    if _NC_CACHE is None:
        _NC_CACHE = build_program()
    return _NC_CACHE


def _make_core_inputs(q, k, v, h, part):
    qbs = [part + NPART * j for j in range(NQ // BLOCK)]
    ls = np.concatenate([np.arange(BLOCK * b, BLOCK * b + BLOCK) for b in qbs])
    qc = q[ls][:, h * G:(h + 1) * G, :].reshape(NQ * G, D)
    qT = np.ascontiguousarray(qc.T)
    kh = k[:, h, :]
    kT = np.ascontiguousarray(kh.T)
    vh = v[:, h, :]
    k16 = np.ascontiguousarray(
        kh.astype(ml_dtypes.bfloat16).reshape(B, 64 * D))
    # duplicated V slabs: row_b[f*128 + p] = V_b[p % 64, f]
    # duplicated V slabs: slab_b[f, p] = V_b[p % 64, f], split by d halves
    vb16 = vh.astype(ml_dtypes.bfloat16).reshape(B, 64, D)
    vsl = np.tile(vb16, (1, 2, 1)).transpose(0, 2, 1)   # [B, 128 f, 128 p]
    v16a = np.ascontiguousarray(vsl[:, 0:64, :].reshape(B, 64 * 128))
    v16b = np.ascontiguousarray(vsl[:, 64:128, :].reshape(B, 64 * 128))

    qb_of_li = ls // BLOCK
    qb_ch = qb_of_li[QPC * np.arange(NCH)]          # qb per chunk
    qbf = np.maximum(qb_ch - 1, 0)

    kabT = np.empty((128, NCH * 128), np.float32)
    vab = np.empty((128, NCH * 128), np.float32)
    for ch in range(NCH):
        kabT[:, 128 * ch:128 * ch + 64] = kT[:, 0:64]
        kabT[:, 128 * ch + 64:128 * ch + 128] = \
            kT[:, 64 * qbf[ch]:64 * qbf[ch] + 64]
        vab[0:64, 128 * ch:128 * ch + 128] = vh[0:64]
        vab[64:128, 128 * ch:128 * ch + 128] = \
            vh[64 * qbf[ch]:64 * qbf[ch] + 64]
    kabT = kabT.astype(ml_dtypes.bfloat16)
    vab = vab.astype(ml_dtypes.bfloat16)

    b2a = np.empty((128, NCH), np.float32)
    b2a[0:64] = np.where(qb_ch >= 1, 0.0, NEG)[None, :]
    b2a[64:128] = np.where(qb_ch >= 2, 0.0, NEG)[None, :]
    b2b = np.broadcast_to(
        np.where(qb_ch >= 3, 0.0, NEG).astype(np.float32), (128, NCH)).copy()

    # stage-1 visibility bias: compressed key c visible iff 16c+31 <= s
    rows_s = ls[(QPC * np.arange(NCH)[None, :] + np.arange(128)[:, None] // G)]
    thr = np.floor((rows_s.astype(np.float64) - (KERNEL - 1)) / STRIDE)
    vis = np.arange(C)[None, :, None] <= thr.T[:, None, :]  # [NCH, C, 128]
    bias1 = np.where(vis, 0.0, NEG)
    bias1 = np.ascontiguousarray(
        bias1.transpose(2, 0, 1).reshape(128, NCH * C)).astype(
            ml_dtypes.bfloat16)

    mmid = np.full((128, NST * B), -1e38, np.float32)
    for sti in range(NST):
        qb_rows = qb_of_li[128 * sti + np.arange(128)]
        allowed = (np.arange(B)[None, :] >= 1) & \
                  (np.arange(B)[None, :] <= qb_rows[:, None] - 2)
        allowed[~allowed.any(axis=1), 1] = True
        mmid[:, B * sti:B * sti + B] = np.where(allowed, 0.0, -1e38)

    iotab = np.broadcast_to(np.arange(B, dtype=np.float32), (128, B)).copy()
    aones48 = np.zeros((128, 48), np.float32)
    for j in range(16):
        aones48[8 * j:8 * j + 8, 16 + j] = 1.0

    onesrow = np.ones((1, 128), ml_dtypes.bfloat16)
    onesb = np.ones((128, 1), ml_dtypes.bfloat16)
    sel128 = (np.arange(128)[:, None] % 16 ==
              np.arange(128)[None, :] % 16).astype(np.float32)
    qmask = (np.arange(128)[:, None] // 16 ==
             np.arange(8)[None, :]).astype(np.float32)

    return {"qT": qT, "qTb": qT.astype(ml_dtypes.bfloat16), "kT": kT,
            "kabT": kabT, "vab": vab, "b2a": b2a, "b2b": b2b, "bias1": bias1,
            "mmid": mmid, "iotab": iotab, "aones48": aones48,
            "onesrow": onesrow, "onesb": onesb, "sel128": sel128,
            "qmask": qmask, "k16": k16, "v16a": v16a, "v16b": v16b,
            "eps20": np.full((128, 1), 1e-20, np.float32)}, ls


def kernel(q, k, v, _profile=False):
    q = np.asarray(q, dtype=np.float32)
    k = np.asarray(k, dtype=np.float32)
    v = np.asarray(v, dtype=np.float32)
    nc = _get_program()

    in_maps = []
    ls_per_core = []
    for c in range(NCORES):
        h, part = divmod(c, NPART)
        im, ls = _make_core_inputs(q, k, v, h, part)
        in_maps.append(im)
        ls_per_core.append(ls)

    kw = dict(trace=True) if _profile else {}
    res = run_bass_kernel_spmd(nc, in_maps, list(range(NCORES)), **kw)

    out = np.zeros((S, HQ, D), dtype=np.float32)
    for c in range(NCORES):
        h, part = divmod(c, NPART)
        oc = res.results[c]["out"]                  # [128 d, NCH*128]
        ocr = oc.reshape(D, NCH, QPC, G).transpose(1, 2, 3, 0)  # [NCH,16,G,D]
        out[ls_per_core[c], h * G:(h + 1) * G, :] = ocr.reshape(NQ, G, D)
    if _profile:
        return out, res
    return out
